# revision 35
# baseline (speedup 1.0000x reference)
"""nn_DirAttention kernel for 8 Trainium2 NeuronCores.

Strategy: data-parallel over batch (B=8, one batch element per core).
Per core, the directional attention

    ah[o,i,j] = sum_k Wc[o,k] * Qh[k,i] * Kh[k,j]   (k = C*L = 4096)

is computed by materialising G[k,(j,i)] = Kh[k,j]*Qh[k,i] per 128-row
k-block on the Vector engine (outer-product broadcast via a
column-duplicated K so every operand presents dense bf16 pairs to the
DVE -> 2x mode), then accumulating ah = Wc' @ G on the PE with even/odd
k-blocks on the two halves of the array.  Softmax over the channel
(partition) axis uses an ACT exp with per-partition bias bc, a
ones-matmul for the column sums, a 64-lane reciprocal via a DRAM
shuffle, and a DMA partition-broadcast of 1/Z.  The 3x3 conv runs as
shifted accumulating matmuls over zero-padded SBUF images, with both
image halves accumulating into one PSUM pass per output chunk.
BatchNorm is folded into the conv weights on the host.
"""

import sys

for _p in ("/opt/trn_rl_repo",):
    if _p not in sys.path:
        sys.path.append(_p)

import numpy as np
import ml_dtypes

import concourse.bacc as bacc
import concourse.bass as bass
import concourse.mybir as mybir
import concourse.tile as tile
from concourse.bass_utils import run_bass_kernel_spmd

BF16 = mybir.dt.bfloat16
F32 = mybir.dt.float32
B, C, L = 8, 64, 64
N = L * L  # 4096
NKB = 32  # 128-row k-blocks in the C*L contraction
BN_EPS = 1e-5
PAD = L + 2  # 66, padded row stride for the conv images

_CACHE = {}


def _build_nc(debug=False):
    nc = bacc.Bacc(target_bir_lowering=False)

    # ---- DRAM parameters -------------------------------------------------
    xbf = nc.dram_tensor("xbf", [C, N], BF16, kind="ExternalInput")
    wqblk = nc.dram_tensor("wqblk", [128, 128], BF16, kind="ExternalInput")
    wkblk = nc.dram_tensor("wkblk", [128, 128], BF16, kind="ExternalInput")
    wcpt = nc.dram_tensor("wcpt", [128, NKB, 64], BF16, kind="ExternalInput")
    woa = nc.dram_tensor("woa", [128, 9, 64], BF16, kind="ExternalInput")
    wob = nc.dram_tensor("wob", [128, 9, 64], BF16, kind="ExternalInput")
    bqq_d = nc.dram_tensor("bqq", [128, 1], F32, kind="ExternalInput")
    bkk_d = nc.dram_tensor("bkk", [128, 1], F32, kind="ExternalInput")
    bc_d = nc.dram_tensor("bc", [64, 1], F32, kind="ExternalInput")
    bo_d = nc.dram_tensor("bo_eff", [64, 1], F32, kind="ExternalInput")
    dv_d = nc.dram_tensor("d_vec", [64, 1], F32, kind="ExternalInput")
    ident_d = nc.dram_tensor("ident", [128, 64], BF16, kind="ExternalInput")
    y = nc.dram_tensor("y", [C, N], F32, kind="ExternalOutput")
    taps = {}
    if debug:
        for nm, shp, dt in [
            ("t_att", [64, N], F32), ("t_z", [64, N], F32),
            ("t_hatt", [64, N], F32), ("t_watt", [64, N], F32),
        ]:
            taps[nm] = nc.dram_tensor(nm, shp, dt, kind="ExternalOutput")

    from contextlib import ExitStack
    with tile.TileContext(nc) as tc, ExitStack() as _es:
        consts = _es.enter_context(tc.tile_pool(name="consts", bufs=1))
        qk = _es.enter_context(tc.tile_pool(name="qk", bufs=1))
        work = _es.enter_context(tc.tile_pool(name="work", bufs=2))
        gpool = _es.enter_context(tc.tile_pool(name="gpool", bufs=16))
        dpool = _es.enter_context(tc.tile_pool(name="dscratch", bufs=2, space="DRAM"))

        # ---- constant loads ---------------------------------------------
        x2h = consts.tile([128, N], BF16)   # rows 0-63: x, 64-127: x shifted 1
        x2w = consts.tile([128, N], BF16)   # rows 0-63: x, 64-127: x shifted 64
        wq_sb = consts.tile([128, 128], BF16)
        wk_sb = consts.tile([128, 128], BF16)
        wc_sb = consts.tile([128, NKB, 64], BF16)
        woa_sb = consts.tile([128, 9, 64], BF16)
        wob_sb = consts.tile([128, 9, 64], BF16)
        bqq = consts.tile([128, 1], F32)
        bkk = consts.tile([128, 1], F32)
        bcv = consts.tile([64, 1], F32)
        bov2 = consts.tile([128, 1], F32)
        dvv2 = consts.tile([128, 1], F32)
        ones = consts.tile([128, 1], BF16)
        ident_sb = consts.tile([128, 64], BF16)

        nc.vector.memset(x2h[64:128, 4088:4096], 0.0)
        nc.vector.memset(x2w[64:128, 4024:4096], 0.0)
        nc.sync.dma_start(out=x2h[0:64, :], in_=xbf[:, :])
        nc.scalar.dma_start(out=x2h[64:128, 0:4095], in_=xbf[:, 1:4096])
        nc.scalar.dma_start(out=x2w[0:64, :], in_=xbf[:, :])
        nc.sync.dma_start(out=x2w[64:128, 0:4032], in_=xbf[:, 64:4096])
        nc.sync.dma_start(out=wq_sb[:], in_=wqblk[:])
        nc.sync.dma_start(out=wk_sb[:], in_=wkblk[:])
        nc.sync.dma_start(out=wc_sb[:], in_=wcpt[:])
        nc.sync.dma_start(out=woa_sb[:], in_=woa[:])
        nc.sync.dma_start(out=wob_sb[:], in_=wob[:])
        nc.sync.dma_start(out=bqq[:], in_=bqq_d[:])
        nc.sync.dma_start(out=bkk[:], in_=bkk_d[:])
        nc.sync.dma_start(out=bcv[:], in_=bc_d[:])
        nc.sync.dma_start(out=bov2[0:64], in_=bo_d[:])
        nc.sync.dma_start(out=bov2[64:128], in_=bo_d[:])
        nc.sync.dma_start(out=dvv2[0:64], in_=dv_d[:])
        nc.sync.dma_start(out=dvv2[64:128], in_=dv_d[:])
        nc.sync.dma_start(out=ident_sb[:], in_=ident_d[:])
        nc.vector.memset(ones[:], 1.0)

        # conv image buffers (zero borders)
        catA = consts.tile([128, PAD * PAD], BF16)  # rows 0-63 x, 64-127 h_att
        catB = consts.tile([128, PAD * PAD], BF16)  # rows 0-63 w_att, rest zero
        for img in (catA, catB):
            # zero only the border: top/bottom pad rows + the fused
            # right/left pad-column pairs of adjacent interior rows
            nc.gpsimd.memset(img[:, 0:PAD], 0.0)
            nc.gpsimd.memset(img[:, (PAD - 1) * PAD:PAD * PAD], 0.0)
            side = bass.AP(tensor=img.tensor, offset=img.offset + (2 * PAD - 1),
                           ap=[img.ap[0], [PAD, PAD - 2], [1, 2]])
            nc.gpsimd.memset(side, 0.0)
        # rows 64-127 of catB never hold data; zero them once for the
        # K=128 conv reads
        nc.gpsimd.memset(catB[64:128, :], 0.0)

        def pad_interior_ap(t, p0, p1, row0=0, nrows=L):
            base = t[p0:p1, :]
            return bass.AP(tensor=base.tensor,
                           offset=base.offset + (row0 + 1) * PAD + 1,
                           ap=[base.ap[0], [PAD, nrows], [1, L]])

        # x part of the conv image, straight from DRAM
        nc.scalar.dma_start(out=pad_interior_ap(catA, 0, 64), in_=xbf[:, :])

        # ---- projections -------------------------------------------------
        # Per direction: Q [128, 32, 64] (block kb = spatial pair, partition
        # = (parity, channel)), Kdup [128, 32, 64, 2] (K duplicated pairs).
        q_t = {d: qk.tile([128, NKB, 64], BF16, tag=f"q{d}", name=f"q_{d}") for d in "hw"}
        kd_t = {d: qk.tile([128, NKB, 64, 2], BF16, tag=f"k{d}", name=f"kd_{d}") for d in "hw"}

        with tc.tile_pool(name="projps", bufs=6, space="PSUM") as pps:
            for d in "hw":
                x2 = x2h if d == "h" else x2w
                for proj in "qk":
                    wsb = wq_sb if proj == "q" else wk_sb
                    bias = bqq if proj == "q" else bkk
                    for t8 in range(4):  # 8 g-blocks per psum tile
                        ps = pps.tile([128, 8, 64], F32, tag="proj")
                        for half in range(2):  # 4 g per matmul
                            g0 = t8 * 8 + half * 4
                            if d == "h":
                                rhs = bass.AP(tensor=x2.tensor, offset=x2.offset + 2 * g0,
                                              ap=[x2.ap[0], [2, 4], [64, 64]])
                            else:
                                rhs = bass.AP(tensor=x2.tensor, offset=x2.offset + 128 * g0,
                                              ap=[x2.ap[0], [128, 4], [1, 64]])
                            nc.tensor.matmul(out=ps[:, half * 4:(half + 1) * 4, :],
                                             lhsT=wsb[:], rhs=rhs,
                                             start=True, stop=True)
                        if proj == "q":
                            nc.scalar.activation(
                                out=q_t[d][:, t8 * 8:(t8 + 1) * 8, :], in_=ps[:],
                                func=mybir.ActivationFunctionType.Identity,
                                bias=bias[:], scale=1.0)
                        else:
                            for dup in range(2):
                                dst = bass.AP(
                                    tensor=kd_t[d].tensor,
                                    offset=kd_t[d].offset + t8 * 8 * 128 + dup,
                                    ap=[kd_t[d].ap[0], [128, 8], [2, 64]])
                                nc.scalar.activation(
                                    out=dst, in_=ps[:],
                                    func=mybir.ActivationFunctionType.Identity,
                                    bias=bias[:], scale=1.0)

        # ---- attention + softmax + apply + conv --------------------------
        # att padded to 128 partitions (rows 64-127 zero) so the Z column
        # sums run as full-width K=128 matmuls.
        att_t = {}
        for d in "hw":
            att_t[d] = work.tile([128, L, L], BF16, tag=f"att{d}", bufs=1, name=f"att_{d}")
            nc.vector.memset(att_t[d][64:128, :, :], 0.0)
        hat_t = {d: work.tile([64, N], BF16, tag=f"hat{d}", bufs=1, name=f"hat_{d}") for d in "hw"}

        def conv_tail(cps, rps):
            # 3x3 conv: both images accumulate into one psum pass per chunk
            for rp in rps:
                cv = cps.tile([128, 512], F32, tag="cv", name="cv")
                for tap in range(9):
                    dy, dx = tap // 3, tap % 3
                    for half in range(2):
                        r = rp * 2 + half
                        off = (r * 8 + dy) * PAD + dx
                        for src_sb, w_sb, part in ((catA, woa_sb, 0), (catB, wob_sb, 1)):
                            rhs = bass.AP(tensor=src_sb.tensor, offset=src_sb.offset + off,
                                          ap=[src_sb.ap[0], [PAD, 8], [1, 64]])
                            nc.tensor.matmul(out=cv[half * 64:(half + 1) * 64, :],
                                             lhsT=w_sb[:, tap, :], rhs=rhs,
                                             start=(tap == 0 and part == 0),
                                             stop=(tap == 8 and part == 1),
                                             skip_group_check=True,
                                             tile_position=(0, half * 64))
                ysb = work.tile([128, 512], F32, tag="ysb", name="ysb")
                nc.scalar.activation(out=ysb[0:64, :], in_=cv[0:64, :],
                                     func=mybir.ActivationFunctionType.Relu,
                                     bias=bov2[0:64], scale=1.0)
                nc.scalar.activation(out=ysb[64:128, :], in_=cv[64:128, :],
                                     func=mybir.ActivationFunctionType.Relu,
                                     bias=bov2[64:128], scale=1.0)
                nc.vector.tensor_scalar_add(out=ysb[:], in0=ysb[:], scalar1=dvv2[:])
                nc.sync.dma_start(out=y[:, (2 * rp) * 512:(2 * rp + 1) * 512],
                                  in_=ysb[0:64, :])
                nc.sync.dma_start(out=y[:, (2 * rp + 1) * 512:(2 * rp + 2) * 512],
                                  in_=ysb[64:128, :])

        with tc.tile_pool(name="ahps", bufs=1, space="PSUM") as aps, \
             tc.tile_pool(name="zps", bufs=2, space="PSUM") as zps, \
             tc.tile_pool(name="cvps", bufs=2, space="PSUM") as cps:
            for d in "hw":
                q, kd = q_t[d], kd_t[d]
                for jh in range(2):  # column halves (j in [jh*32, jh*32+32))
                    ah = aps.tile([128, 2048], F32, tag="ah", name="ah")
                    for kbp in range(NKB // 2):
                        grhs = {}
                        for half in range(2):
                            kb = kbp * 2 + half
                            g = gpool.tile([128, 32, 64], BF16, tag="g", name=f"g{half}")
                            # G[k, j, i] = K[k,j] * Q[k,i] (2x-mode paired APs)
                            in0 = bass.AP(
                                tensor=kd.tensor,
                                offset=kd.offset + kb * 128 + jh * 64,
                                ap=[kd.ap[0], [2, 32], [0, 32], [1, 2]])
                            in1 = bass.AP(
                                tensor=q.tensor, offset=q.offset + kb * 64,
                                ap=[q.ap[0], [0, 32], [2, 32], [1, 2]])
                            gout = bass.AP(
                                tensor=g.tensor, offset=g.offset,
                                ap=[g.ap[0], [64, 32], [2, 32], [1, 2]])
                            nc.vector.tensor_mul(out=gout, in0=in0, in1=in1)
                            grhs[half] = g[:].rearrange("p a b -> p (a b)")
                        for ns in range(4):
                            for half in range(2):
                                kb = kbp * 2 + half
                                nc.tensor.matmul(
                                    out=ah[half * 64:(half + 1) * 64, ns * 512:(ns + 1) * 512],
                                    lhsT=wc_sb[:, kb, :],
                                    rhs=grhs[half][:, ns * 512:(ns + 1) * 512],
                                    start=(kbp == 0),
                                    stop=(kbp == NKB // 2 - 1 and ns == 3),
                                    skip_group_check=True,
                                    tile_position=(0, half * 64))
                    # fold the odd-half partial into the even-half region via
                    # an identity matmul (ACT copy to SBUF, same partitions).
                    fold = work.tile([128, 2048], BF16, tag="fold", name="fold", bufs=2)
                    nc.scalar.copy(out=fold[64:128, :], in_=ah[64:128, :])
                    for ns in range(4):
                        nc.tensor.matmul(
                            out=ah[0:64, ns * 512:(ns + 1) * 512],
                            lhsT=ident_sb[64:128, :],
                            rhs=fold[64:128, ns * 512:(ns + 1) * 512],
                            start=False, stop=True,
                            skip_group_check=True,
                            tile_position=(64, 0))
                    # exp with transposed read: ah[(j,i)] -> att[(i, j)]
                    src = bass.AP(tensor=ah.tensor, offset=ah.offset,
                                  ap=[[ah.ap[0][0], 64], [1, 64], [64, 32]])
                    nc.scalar.activation(
                        out=att_t[d][0:64, :, jh * 32:(jh + 1) * 32], in_=src,
                        func=mybir.ActivationFunctionType.Exp,
                        bias=bcv[:], scale=1.0)

                att = att_t[d][:].rearrange("p a b -> p (a b)")
                if debug:
                    tp = taps["t_att"] if d == "h" else taps["t_z"]
                    nc.sync.dma_start(out=tp[:], in_=att[0:64, :])
                # column sums Z via ones-matmul (K=128; att rows 64-127 zero),
                # then spread Z across 64 lanes via DRAM for the reciprocal
                zrow = work.tile([1, N], F32, tag="zrow", bufs=1)
                for col in range(8):
                    zt = zps.tile([1, 512], F32, tag="z", name="zt")
                    nc.tensor.matmul(out=zt[:], lhsT=ones[:],
                                     rhs=att[:, col * 512:(col + 1) * 512],
                                     start=True, stop=True)
                    nc.scalar.copy(out=zrow[:, col * 512:(col + 1) * 512], in_=zt[:])
                zs = work.tile([64, 64], F32, tag="zs", bufs=1)
                rzs = work.tile([64, 64], BF16, tag="rzs", bufs=1)
                nc.sync.dma_start(out=zs[:], in_=zrow[:])
                with nc.allow_low_precision(reason="1/Z multiplier in bf16"):
                    nc.vector.reciprocal(out=rzs[:], in_=zs[:])
                rz = dpool.tile([64, 64], BF16, tag="rzd2")
                nc.sync.dma_start(out=rz[:], in_=rzs[:])
                rzb = work.tile([64, N], BF16, tag="rzb", bufs=1)
                for ch in range(2):
                    sl = slice(ch * 2048, (ch + 1) * 2048)
                    nc.sync.dma_start(
                        out=rzb[:, sl],
                        in_=bass.AP(tensor=rz.tensor, offset=rz.offset + ch * 2048,
                                    ap=[[0, 64], [64, 32], [1, 64]]))
                # h_att = x * att * (1/Z); write into the padded conv image
                tmp = work.tile([64, N], BF16, tag="tmp", bufs=1)
                nc.vector.tensor_mul(out=tmp[:], in0=att[0:64, :], in1=x2h[0:64, :])
                cat_dst, cat_p0 = (catA, 64) if d == "h" else (catB, 0)
                if d == "h":
                    for ch in range(2):
                        sl = slice(ch * 2048, (ch + 1) * 2048)
                        nc.vector.tensor_mul(out=hat_t[d][:, sl], in0=tmp[:, sl],
                                             in1=rzb[:, sl])
                        nc.sync.dma_start(
                            out=pad_interior_ap(cat_dst, cat_p0, cat_p0 + 64,
                                                row0=ch * 32, nrows=32),
                            in_=hat_t[d][:, sl])
                else:
                    # 4 row-band chunks, written straight into the padded
                    # image (strided dst); conv row-pair rp needs image rows
                    # up to 16(rp+1), so emit conv rp-1 after each chunk to
                    # pipeline the conv with the apply.
                    for ch in range(4):
                        sl = slice(ch * 1024, (ch + 1) * 1024)
                        tv = tmp[:, sl].rearrange("p (a b) -> p a b", b=64)
                        rv = rzb[:, sl].rearrange("p (a b) -> p a b", b=64)
                        nc.vector.tensor_mul(
                            out=pad_interior_ap(cat_dst, cat_p0, cat_p0 + 64,
                                                row0=ch * 16, nrows=16),
                            in0=tv, in1=rv)
                        if ch >= 1:
                            conv_tail(cps, [ch - 1])
                    conv_tail(cps, [3])

        if debug:
            nc.sync.dma_start(out=taps["t_hatt"][:], in_=hat_t["h"][:])
            nc.sync.dma_start(out=taps["t_watt"][:], in_=hat_t["w"][:])

    nc.finalize()
    return nc


def _host_prep(Wq, bq, Wk, bk, Wc, bc, Wo, bo, gamma, beta, run_mean, run_var):
    bf = ml_dtypes.bfloat16
    wqblk = np.zeros((128, 128), np.float32)
    wqblk[0:64, 0:64] = Wq.T
    wqblk[64:128, 64:128] = Wq.T
    wkblk = np.zeros((128, 128), np.float32)
    wkblk[0:64, 0:64] = Wk.T
    wkblk[64:128, 64:128] = Wk.T
    # Wc permuted so the contraction index is (spatial, channel)
    wcp = Wc.reshape(C, C, L).transpose(0, 2, 1).reshape(C, C * L)
    wcpt = np.ascontiguousarray(
        wcp.T.reshape(NKB, 128, 64).transpose(1, 0, 2))  # [128, 32, 64]
    inv = gamma / np.sqrt(run_var + BN_EPS)
    wo_eff = Wo * inv[:, None, None, None]
    wot = wo_eff.transpose(1, 2, 3, 0).reshape(3 * C, 9, C)  # [192, 9, 64]
    zpad = np.zeros((64, 9, C), np.float32)
    return {
        "wqblk": wqblk.astype(bf), "wkblk": wkblk.astype(bf),
        "wcpt": wcpt.astype(bf),
        "woa": np.ascontiguousarray(wot[0:128]).astype(bf),
        "wob": np.concatenate([wot[128:192], zpad]).astype(bf),
        "bqq": np.concatenate([bq, bq]).reshape(128, 1).astype(np.float32),
        "bkk": np.concatenate([bk, bk]).reshape(128, 1).astype(np.float32),
        "bc": bc.reshape(64, 1).astype(np.float32),
        "bo_eff": (bo * inv).reshape(64, 1).astype(np.float32),
        "d_vec": (beta - run_mean * inv).reshape(64, 1).astype(np.float32),
        "ident": np.concatenate([np.zeros((64, 64), np.float32),
                                 np.eye(64, dtype=np.float32)]).astype(bf),
    }


def kernel(x, Wq, bq, Wk, bk, Wc, bc, Wo, bo, gamma, beta, run_mean, run_var,
           debug=False, trace=False, trace_kwargs=None):
    x = np.asarray(x, np.float32)
    weights = _host_prep(
        np.asarray(Wq, np.float32), np.asarray(bq, np.float32),
        np.asarray(Wk, np.float32), np.asarray(bk, np.float32),
        np.asarray(Wc, np.float32), np.asarray(bc, np.float32),
        np.asarray(Wo, np.float32), np.asarray(bo, np.float32),
        np.asarray(gamma, np.float32), np.asarray(beta, np.float32),
        np.asarray(run_mean, np.float32), np.asarray(run_var, np.float32))
    key = bool(debug)
    if key not in _CACHE:
        _CACHE[key] = _build_nc(debug=debug)
    nc = _CACHE[key]
    bf = ml_dtypes.bfloat16
    in_maps = []
    for b in range(B):
        m = dict(weights)
        m["xbf"] = np.ascontiguousarray(x[b].reshape(C, N)).astype(bf)
        in_maps.append(m)
    kwargs = {}
    if trace:
        kwargs = dict(trace=True, trace_cores=[0], **(trace_kwargs or {}))
    res = run_bass_kernel_spmd(nc, in_maps, core_ids=list(range(B)), **kwargs)
    out = np.stack([res.results[b]["y"].reshape(C, L, L) for b in range(B)])
    if debug or trace:
        return out, res
    return out


# revision 36
# speedup vs baseline: 1.0481x; 1.0481x over previous
"""nn_DirAttention kernel for 8 Trainium2 NeuronCores.

Strategy: data-parallel over batch (B=8, one batch element per core).
Per core, the directional attention

    ah[o,i,j] = sum_k Wc[o,k] * Qh[k,i] * Kh[k,j]   (k = C*L = 4096)

is computed by materialising G[k,(j,i)] = Kh[k,j]*Qh[k,i] per 128-row
k-block on the Vector engine (outer-product broadcast via a
column-duplicated K so every operand presents dense bf16 pairs to the
DVE -> 2x mode), then accumulating ah = Wc' @ G on the PE with even/odd
k-blocks on the two halves of the array.  Softmax over the channel
(partition) axis uses an ACT exp with per-partition bias bc, a
ones-matmul for the column sums, a 64-lane reciprocal via a DRAM
shuffle, and a DMA partition-broadcast of 1/Z.  The 3x3 conv runs as
shifted accumulating matmuls over zero-padded SBUF images, with both
image halves accumulating into one PSUM pass per output chunk.
BatchNorm is folded into the conv weights on the host.
"""

import sys

for _p in ("/opt/trn_rl_repo",):
    if _p not in sys.path:
        sys.path.append(_p)

import numpy as np
import ml_dtypes

import concourse.bacc as bacc
import concourse.bass as bass
import concourse.mybir as mybir
import concourse.tile as tile
from concourse.bass_utils import run_bass_kernel_spmd

BF16 = mybir.dt.bfloat16
F32 = mybir.dt.float32
B, C, L = 8, 64, 64
N = L * L  # 4096
NKB = 32  # 128-row k-blocks in the C*L contraction
BN_EPS = 1e-5
PAD = L + 2  # 66, padded row stride for the conv images

_CACHE = {}


def _build_nc(debug=False):
    nc = bacc.Bacc(target_bir_lowering=False)

    # ---- DRAM parameters -------------------------------------------------
    xbf = nc.dram_tensor("xbf", [C, N], BF16, kind="ExternalInput")
    wqblk = nc.dram_tensor("wqblk", [128, 128], BF16, kind="ExternalInput")
    wkblk = nc.dram_tensor("wkblk", [128, 128], BF16, kind="ExternalInput")
    wcpt = nc.dram_tensor("wcpt", [128, NKB, 64], BF16, kind="ExternalInput")
    woa = nc.dram_tensor("woa", [128, 9, 64], BF16, kind="ExternalInput")
    wob = nc.dram_tensor("wob", [128, 9, 64], BF16, kind="ExternalInput")
    bqq_d = nc.dram_tensor("bqq", [128, 1], F32, kind="ExternalInput")
    bkk_d = nc.dram_tensor("bkk", [128, 1], F32, kind="ExternalInput")
    bc_d = nc.dram_tensor("bc", [64, 1], F32, kind="ExternalInput")
    bo_d = nc.dram_tensor("bo_eff", [64, 1], F32, kind="ExternalInput")
    dv_d = nc.dram_tensor("d_vec", [64, 1], F32, kind="ExternalInput")
    ident_d = nc.dram_tensor("ident", [128, 64], BF16, kind="ExternalInput")
    y = nc.dram_tensor("y", [C, N], F32, kind="ExternalOutput")
    taps = {}
    if debug:
        for nm, shp, dt in [
            ("t_att", [64, N], F32), ("t_z", [64, N], F32),
            ("t_hatt", [64, N], F32), ("t_watt", [64, N], F32),
        ]:
            taps[nm] = nc.dram_tensor(nm, shp, dt, kind="ExternalOutput")

    from contextlib import ExitStack
    with tile.TileContext(nc) as tc, ExitStack() as _es:
        consts = _es.enter_context(tc.tile_pool(name="consts", bufs=1))
        qk = _es.enter_context(tc.tile_pool(name="qk", bufs=1))
        work = _es.enter_context(tc.tile_pool(name="work", bufs=2))
        gpool = _es.enter_context(tc.tile_pool(name="gpool", bufs=16))
        dpool = _es.enter_context(tc.tile_pool(name="dscratch", bufs=2, space="DRAM"))

        # ---- constant loads ---------------------------------------------
        x2h = consts.tile([128, N], BF16)   # rows 0-63: x, 64-127: x shifted 1
        x2w = consts.tile([128, N], BF16)   # rows 0-63: x, 64-127: x shifted 64
        wq_sb = consts.tile([128, 128], BF16)
        wk_sb = consts.tile([128, 128], BF16)
        wc_sb = consts.tile([128, NKB, 64], BF16)
        woa_sb = consts.tile([128, 9, 64], BF16)
        wob_sb = consts.tile([128, 9, 64], BF16)
        bqq = consts.tile([128, 1], F32)
        bkk = consts.tile([128, 1], F32)
        bcv = consts.tile([64, 1], F32)
        bov2 = consts.tile([128, 1], F32)
        dvv2 = consts.tile([128, 1], F32)
        ones = consts.tile([128, 1], BF16)
        ident_sb = consts.tile([128, 64], BF16)

        nc.vector.memset(x2h[64:128, 4088:4096], 0.0)
        nc.vector.memset(x2w[64:128, 4024:4096], 0.0)
        nc.sync.dma_start(out=x2h[0:64, :], in_=xbf[:, :])
        nc.scalar.dma_start(out=x2h[64:128, 0:4095], in_=xbf[:, 1:4096])
        nc.scalar.dma_start(out=x2w[0:64, :], in_=xbf[:, :])
        nc.sync.dma_start(out=x2w[64:128, 0:4032], in_=xbf[:, 64:4096])
        nc.sync.dma_start(out=wq_sb[:], in_=wqblk[:])
        nc.sync.dma_start(out=wk_sb[:], in_=wkblk[:])
        nc.sync.dma_start(out=wc_sb[:], in_=wcpt[:])
        nc.sync.dma_start(out=woa_sb[:], in_=woa[:])
        nc.sync.dma_start(out=wob_sb[:], in_=wob[:])
        nc.sync.dma_start(out=bqq[:], in_=bqq_d[:])
        nc.sync.dma_start(out=bkk[:], in_=bkk_d[:])
        nc.sync.dma_start(out=bcv[:], in_=bc_d[:])
        nc.sync.dma_start(out=bov2[0:64], in_=bo_d[:])
        nc.sync.dma_start(out=bov2[64:128], in_=bo_d[:])
        nc.sync.dma_start(out=dvv2[0:64], in_=dv_d[:])
        nc.sync.dma_start(out=dvv2[64:128], in_=dv_d[:])
        nc.sync.dma_start(out=ident_sb[:], in_=ident_d[:])
        nc.vector.memset(ones[:], 1.0)

        # conv image buffers (zero borders)
        catA = consts.tile([128, PAD * PAD], BF16)  # rows 0-63 x, 64-127 h_att
        catB = consts.tile([128, PAD * PAD], BF16)  # rows 0-63 w_att, rest zero
        nc.gpsimd.memset(catA[:], 0.0)
        nc.gpsimd.memset(catB[:], 0.0)

        def pad_interior_ap(t, p0, p1, row0=0, nrows=L):
            base = t[p0:p1, :]
            return bass.AP(tensor=base.tensor,
                           offset=base.offset + (row0 + 1) * PAD + 1,
                           ap=[base.ap[0], [PAD, nrows], [1, L]])

        # x part of the conv image
        nc.sync.dma_start(out=pad_interior_ap(catA, 0, 64), in_=x2h[0:64, :])

        # ---- projections -------------------------------------------------
        # Per direction: Q [128, 32, 64] (block kb = spatial pair, partition
        # = (parity, channel)), Kdup [128, 32, 64, 2] (K duplicated pairs).
        q_t = {d: qk.tile([128, NKB, 64], BF16, tag=f"q{d}", name=f"q_{d}") for d in "hw"}
        kd_t = {d: qk.tile([128, NKB, 64, 2], BF16, tag=f"k{d}", name=f"kd_{d}") for d in "hw"}

        with tc.tile_pool(name="projps", bufs=6, space="PSUM") as pps:
            for d in "hw":
                x2 = x2h if d == "h" else x2w
                for proj in "qk":
                    wsb = wq_sb if proj == "q" else wk_sb
                    bias = bqq if proj == "q" else bkk
                    for t8 in range(4):  # 8 g-blocks per psum tile
                        ps = pps.tile([128, 8, 64], F32, tag="proj")
                        for half in range(2):  # 4 g per matmul
                            g0 = t8 * 8 + half * 4
                            if d == "h":
                                rhs = bass.AP(tensor=x2.tensor, offset=x2.offset + 2 * g0,
                                              ap=[x2.ap[0], [2, 4], [64, 64]])
                            else:
                                rhs = bass.AP(tensor=x2.tensor, offset=x2.offset + 128 * g0,
                                              ap=[x2.ap[0], [128, 4], [1, 64]])
                            nc.tensor.matmul(out=ps[:, half * 4:(half + 1) * 4, :],
                                             lhsT=wsb[:], rhs=rhs,
                                             start=True, stop=True)
                        if proj == "q":
                            nc.scalar.activation(
                                out=q_t[d][:, t8 * 8:(t8 + 1) * 8, :], in_=ps[:],
                                func=mybir.ActivationFunctionType.Identity,
                                bias=bias[:], scale=1.0)
                        else:
                            for dup in range(2):
                                dst = bass.AP(
                                    tensor=kd_t[d].tensor,
                                    offset=kd_t[d].offset + t8 * 8 * 128 + dup,
                                    ap=[kd_t[d].ap[0], [128, 8], [2, 64]])
                                nc.scalar.activation(
                                    out=dst, in_=ps[:],
                                    func=mybir.ActivationFunctionType.Identity,
                                    bias=bias[:], scale=1.0)

        # ---- attention + softmax + apply + conv --------------------------
        # att padded to 128 partitions (rows 64-127 zero) so the Z column
        # sums run as full-width K=128 matmuls.
        att_t = {}
        for d in "hw":
            att_t[d] = work.tile([128, L, L], BF16, tag=f"att{d}", bufs=1, name=f"att_{d}")
            nc.vector.memset(att_t[d][64:128, :, :], 0.0)
        hat_t = {d: work.tile([64, N], BF16, tag=f"hat{d}", bufs=1, name=f"hat_{d}") for d in "hw"}

        def conv_tail(cps, rps):
            # 3x3 conv: both images accumulate into one psum pass per chunk
            for rp in rps:
                cv = cps.tile([128, 512], F32, tag="cv", name="cv")
                for tap in range(9):
                    dy, dx = tap // 3, tap % 3
                    for half in range(2):
                        r = rp * 2 + half
                        off = (r * 8 + dy) * PAD + dx
                        for src_sb, w_sb, part in ((catA, woa_sb, 0), (catB, wob_sb, 1)):
                            rhs = bass.AP(tensor=src_sb.tensor, offset=src_sb.offset + off,
                                          ap=[src_sb.ap[0], [PAD, 8], [1, 64]])
                            nc.tensor.matmul(out=cv[half * 64:(half + 1) * 64, :],
                                             lhsT=w_sb[:, tap, :], rhs=rhs,
                                             start=(tap == 0 and part == 0),
                                             stop=(tap == 8 and part == 1),
                                             skip_group_check=True,
                                             tile_position=(0, half * 64))
                ysb = work.tile([128, 512], F32, tag="ysb", name="ysb")
                nc.scalar.activation(out=ysb[0:64, :], in_=cv[0:64, :],
                                     func=mybir.ActivationFunctionType.Relu,
                                     bias=bov2[0:64], scale=1.0)
                nc.scalar.activation(out=ysb[64:128, :], in_=cv[64:128, :],
                                     func=mybir.ActivationFunctionType.Relu,
                                     bias=bov2[64:128], scale=1.0)
                nc.vector.tensor_scalar_add(out=ysb[:], in0=ysb[:], scalar1=dvv2[:])
                nc.sync.dma_start(out=y[:, (2 * rp) * 512:(2 * rp + 1) * 512],
                                  in_=ysb[0:64, :])
                nc.sync.dma_start(out=y[:, (2 * rp + 1) * 512:(2 * rp + 2) * 512],
                                  in_=ysb[64:128, :])

        with tc.tile_pool(name="ahps", bufs=1, space="PSUM") as aps, \
             tc.tile_pool(name="zps", bufs=2, space="PSUM") as zps, \
             tc.tile_pool(name="cvps", bufs=2, space="PSUM") as cps:
            for d in "hw":
                q, kd = q_t[d], kd_t[d]
                for jh in range(2):  # column halves (j in [jh*32, jh*32+32))
                    ah = aps.tile([128, 2048], F32, tag="ah", name="ah")
                    for kbp in range(NKB // 2):
                        grhs = {}
                        for half in range(2):
                            kb = kbp * 2 + half
                            g = gpool.tile([128, 32, 64], BF16, tag="g", name=f"g{half}")
                            # G[k, j, i] = K[k,j] * Q[k,i] (2x-mode paired APs)
                            in0 = bass.AP(
                                tensor=kd.tensor,
                                offset=kd.offset + kb * 128 + jh * 64,
                                ap=[kd.ap[0], [2, 32], [0, 32], [1, 2]])
                            in1 = bass.AP(
                                tensor=q.tensor, offset=q.offset + kb * 64,
                                ap=[q.ap[0], [0, 32], [2, 32], [1, 2]])
                            gout = bass.AP(
                                tensor=g.tensor, offset=g.offset,
                                ap=[g.ap[0], [64, 32], [2, 32], [1, 2]])
                            nc.vector.tensor_mul(out=gout, in0=in0, in1=in1)
                            grhs[half] = g[:].rearrange("p a b -> p (a b)")
                        for ns in range(4):
                            for half in range(2):
                                kb = kbp * 2 + half
                                nc.tensor.matmul(
                                    out=ah[half * 64:(half + 1) * 64, ns * 512:(ns + 1) * 512],
                                    lhsT=wc_sb[:, kb, :],
                                    rhs=grhs[half][:, ns * 512:(ns + 1) * 512],
                                    start=(kbp == 0),
                                    stop=(kbp == NKB // 2 - 1 and ns == 3),
                                    skip_group_check=True,
                                    tile_position=(0, half * 64))
                    # fold the odd-half partial into the even-half region via
                    # an identity matmul (ACT copy to SBUF, same partitions).
                    fold = work.tile([128, 2048], BF16, tag="fold", name="fold", bufs=2)
                    nc.scalar.copy(out=fold[64:128, :], in_=ah[64:128, :])
                    for ns in range(4):
                        nc.tensor.matmul(
                            out=ah[0:64, ns * 512:(ns + 1) * 512],
                            lhsT=ident_sb[64:128, :],
                            rhs=fold[64:128, ns * 512:(ns + 1) * 512],
                            start=False, stop=True,
                            skip_group_check=True,
                            tile_position=(64, 0))
                    # exp with transposed read: ah[(j,i)] -> att[(i, j)]
                    src = bass.AP(tensor=ah.tensor, offset=ah.offset,
                                  ap=[[ah.ap[0][0], 64], [1, 64], [64, 32]])
                    nc.scalar.activation(
                        out=att_t[d][0:64, :, jh * 32:(jh + 1) * 32], in_=src,
                        func=mybir.ActivationFunctionType.Exp,
                        bias=bcv[:], scale=1.0)

                att = att_t[d][:].rearrange("p a b -> p (a b)")
                if debug:
                    tp = taps["t_att"] if d == "h" else taps["t_z"]
                    nc.sync.dma_start(out=tp[:], in_=att[0:64, :])
                # column sums Z via ones-matmul (K=128; att rows 64-127 zero),
                # then spread Z across 64 lanes via DRAM for the reciprocal
                zrow = work.tile([1, N], F32, tag="zrow", bufs=1)
                for col in range(8):
                    zt = zps.tile([1, 512], F32, tag="z", name="zt")
                    nc.tensor.matmul(out=zt[:], lhsT=ones[:],
                                     rhs=att[:, col * 512:(col + 1) * 512],
                                     start=True, stop=True)
                    nc.scalar.copy(out=zrow[:, col * 512:(col + 1) * 512], in_=zt[:])
                zs = work.tile([64, 64], F32, tag="zs", bufs=1)
                rzs = work.tile([64, 64], BF16, tag="rzs", bufs=1)
                nc.sync.dma_start(out=zs[:], in_=zrow[:])
                with nc.allow_low_precision(reason="1/Z multiplier in bf16"):
                    nc.vector.reciprocal(out=rzs[:], in_=zs[:])
                rz = dpool.tile([64, 64], BF16, tag="rzd2")
                nc.sync.dma_start(out=rz[:], in_=rzs[:])
                rzb = work.tile([64, N], BF16, tag="rzb", bufs=1)
                for ch in range(2):
                    sl = slice(ch * 2048, (ch + 1) * 2048)
                    nc.sync.dma_start(
                        out=rzb[:, sl],
                        in_=bass.AP(tensor=rz.tensor, offset=rz.offset + ch * 2048,
                                    ap=[[0, 64], [64, 32], [1, 64]]))
                # h_att = x * att * (1/Z); write into the padded conv image
                tmp = work.tile([64, N], BF16, tag="tmp", bufs=1)
                nc.vector.tensor_mul(out=tmp[:], in0=att[0:64, :], in1=x2h[0:64, :])
                cat_dst, cat_p0 = (catA, 64) if d == "h" else (catB, 0)
                if d == "h":
                    for ch in range(2):
                        sl = slice(ch * 2048, (ch + 1) * 2048)
                        nc.vector.tensor_mul(out=hat_t[d][:, sl], in0=tmp[:, sl],
                                             in1=rzb[:, sl])
                        nc.sync.dma_start(
                            out=pad_interior_ap(cat_dst, cat_p0, cat_p0 + 64,
                                                row0=ch * 32, nrows=32),
                            in_=hat_t[d][:, sl])
                else:
                    # 4 row-band chunks, written straight into the padded
                    # image (strided dst); conv row-pair rp needs image rows
                    # up to 16(rp+1), so emit conv rp-1 after each chunk to
                    # pipeline the conv with the apply.
                    for ch in range(4):
                        sl = slice(ch * 1024, (ch + 1) * 1024)
                        tv = tmp[:, sl].rearrange("p (a b) -> p a b", b=64)
                        rv = rzb[:, sl].rearrange("p (a b) -> p a b", b=64)
                        nc.vector.tensor_mul(
                            out=pad_interior_ap(cat_dst, cat_p0, cat_p0 + 64,
                                                row0=ch * 16, nrows=16),
                            in0=tv, in1=rv)
                        if ch >= 1:
                            conv_tail(cps, [ch - 1])
                    conv_tail(cps, [3])

        if debug:
            nc.sync.dma_start(out=taps["t_hatt"][:], in_=hat_t["h"][:])
            nc.sync.dma_start(out=taps["t_watt"][:], in_=hat_t["w"][:])

    nc.finalize()
    return nc


def _host_prep(Wq, bq, Wk, bk, Wc, bc, Wo, bo, gamma, beta, run_mean, run_var):
    bf = ml_dtypes.bfloat16
    wqblk = np.zeros((128, 128), np.float32)
    wqblk[0:64, 0:64] = Wq.T
    wqblk[64:128, 64:128] = Wq.T
    wkblk = np.zeros((128, 128), np.float32)
    wkblk[0:64, 0:64] = Wk.T
    wkblk[64:128, 64:128] = Wk.T
    # Wc permuted so the contraction index is (spatial, channel)
    wcp = Wc.reshape(C, C, L).transpose(0, 2, 1).reshape(C, C * L)
    wcpt = np.ascontiguousarray(
        wcp.T.reshape(NKB, 128, 64).transpose(1, 0, 2))  # [128, 32, 64]
    inv = gamma / np.sqrt(run_var + BN_EPS)
    wo_eff = Wo * inv[:, None, None, None]
    wot = wo_eff.transpose(1, 2, 3, 0).reshape(3 * C, 9, C)  # [192, 9, 64]
    zpad = np.zeros((64, 9, C), np.float32)
    return {
        "wqblk": wqblk.astype(bf), "wkblk": wkblk.astype(bf),
        "wcpt": wcpt.astype(bf),
        "woa": np.ascontiguousarray(wot[0:128]).astype(bf),
        "wob": np.concatenate([wot[128:192], zpad]).astype(bf),
        "bqq": np.concatenate([bq, bq]).reshape(128, 1).astype(np.float32),
        "bkk": np.concatenate([bk, bk]).reshape(128, 1).astype(np.float32),
        "bc": bc.reshape(64, 1).astype(np.float32),
        "bo_eff": (bo * inv).reshape(64, 1).astype(np.float32),
        "d_vec": (beta - run_mean * inv).reshape(64, 1).astype(np.float32),
        "ident": np.concatenate([np.zeros((64, 64), np.float32),
                                 np.eye(64, dtype=np.float32)]).astype(bf),
    }


def kernel(x, Wq, bq, Wk, bk, Wc, bc, Wo, bo, gamma, beta, run_mean, run_var,
           debug=False, trace=False, trace_kwargs=None):
    x = np.asarray(x, np.float32)
    weights = _host_prep(
        np.asarray(Wq, np.float32), np.asarray(bq, np.float32),
        np.asarray(Wk, np.float32), np.asarray(bk, np.float32),
        np.asarray(Wc, np.float32), np.asarray(bc, np.float32),
        np.asarray(Wo, np.float32), np.asarray(bo, np.float32),
        np.asarray(gamma, np.float32), np.asarray(beta, np.float32),
        np.asarray(run_mean, np.float32), np.asarray(run_var, np.float32))
    key = bool(debug)
    if key not in _CACHE:
        _CACHE[key] = _build_nc(debug=debug)
    nc = _CACHE[key]
    bf = ml_dtypes.bfloat16
    in_maps = []
    for b in range(B):
        m = dict(weights)
        m["xbf"] = np.ascontiguousarray(x[b].reshape(C, N)).astype(bf)
        in_maps.append(m)
    kwargs = {}
    if trace:
        kwargs = dict(trace=True, trace_cores=[0], **(trace_kwargs or {}))
    res = run_bass_kernel_spmd(nc, in_maps, core_ids=list(range(B)), **kwargs)
    out = np.stack([res.results[b]["y"].reshape(C, L, L) for b in range(B)])
    if debug or trace:
        return out, res
    return out


# revision 37
# speedup vs baseline: 1.0875x; 1.0377x over previous
"""nn_DirAttention kernel for 8 Trainium2 NeuronCores.

Strategy: data-parallel over batch (B=8, one batch element per core).
Per core, the directional attention

    ah[o,i,j] = sum_k Wc[o,k] * Qh[k,i] * Kh[k,j]   (k = C*L = 4096)

is computed by materialising G[k,(j,i)] = Kh[k,j]*Qh[k,i] per 128-row
k-block on the Vector engine (outer-product broadcast via a
column-duplicated K so every operand presents dense bf16 pairs to the
DVE -> 2x mode), then accumulating ah = Wc' @ G on the PE with even/odd
k-blocks on the two halves of the array.  Softmax over the channel
(partition) axis uses an ACT exp with per-partition bias bc, a
ones-matmul for the column sums, a 64-lane reciprocal via a DRAM
shuffle, and a DMA partition-broadcast of 1/Z.  The 3x3 conv runs as
shifted accumulating matmuls over zero-padded SBUF images, with both
image halves accumulating into one PSUM pass per output chunk.
BatchNorm is folded into the conv weights on the host.
"""

import sys

for _p in ("/opt/trn_rl_repo",):
    if _p not in sys.path:
        sys.path.append(_p)

import numpy as np
import ml_dtypes

import concourse.bacc as bacc
import concourse.bass as bass
import concourse.mybir as mybir
import concourse.tile as tile
from concourse.bass_utils import run_bass_kernel_spmd

BF16 = mybir.dt.bfloat16
F32 = mybir.dt.float32
B, C, L = 8, 64, 64
N = L * L  # 4096
NKB = 32  # 128-row k-blocks in the C*L contraction
BN_EPS = 1e-5
PAD = L + 2  # 66, padded row stride for the conv images

_CACHE = {}


def _build_nc(debug=False):
    nc = bacc.Bacc(target_bir_lowering=False)

    # ---- DRAM parameters -------------------------------------------------
    xbf = nc.dram_tensor("xbf", [C, N], BF16, kind="ExternalInput")
    wqblk = nc.dram_tensor("wqblk", [128, 128], BF16, kind="ExternalInput")
    wkblk = nc.dram_tensor("wkblk", [128, 128], BF16, kind="ExternalInput")
    wcpt = nc.dram_tensor("wcpt", [128, NKB, 64], BF16, kind="ExternalInput")
    woa = nc.dram_tensor("woa", [128, 9, 64], BF16, kind="ExternalInput")
    wob = nc.dram_tensor("wob", [128, 9, 64], BF16, kind="ExternalInput")
    bqq_d = nc.dram_tensor("bqq", [128, 1], F32, kind="ExternalInput")
    bkk_d = nc.dram_tensor("bkk", [128, 1], F32, kind="ExternalInput")
    bc_d = nc.dram_tensor("bc", [64, 1], F32, kind="ExternalInput")
    bo_d = nc.dram_tensor("bo_eff", [64, 1], F32, kind="ExternalInput")
    dv_d = nc.dram_tensor("d_vec", [64, 1], F32, kind="ExternalInput")
    ident_d = nc.dram_tensor("ident", [128, 64], BF16, kind="ExternalInput")
    y = nc.dram_tensor("y", [C, N], F32, kind="ExternalOutput")
    taps = {}
    if debug:
        for nm, shp, dt in [
            ("t_att", [64, N], F32), ("t_z", [64, N], F32),
            ("t_hatt", [64, N], F32), ("t_watt", [64, N], F32),
        ]:
            taps[nm] = nc.dram_tensor(nm, shp, dt, kind="ExternalOutput")

    from contextlib import ExitStack
    with tile.TileContext(nc) as tc, ExitStack() as _es:
        consts = _es.enter_context(tc.tile_pool(name="consts", bufs=1))
        qk = _es.enter_context(tc.tile_pool(name="qk", bufs=1))
        work = _es.enter_context(tc.tile_pool(name="work", bufs=2))
        gpool = _es.enter_context(tc.tile_pool(name="gpool", bufs=16))
        dpool = _es.enter_context(tc.tile_pool(name="dscratch", bufs=2, space="DRAM"))

        # ---- constant loads ---------------------------------------------
        x2h = consts.tile([128, N], BF16)   # rows 0-63: x, 64-127: x shifted 1
        x2w = consts.tile([128, N], BF16)   # rows 0-63: x, 64-127: x shifted 64
        wq_sb = consts.tile([128, 128], BF16)
        wk_sb = consts.tile([128, 128], BF16)
        wc_sb = consts.tile([128, NKB, 64], BF16)
        woa_sb = consts.tile([128, 9, 64], BF16)
        wob_sb = consts.tile([128, 9, 64], BF16)
        bqq = consts.tile([128, 1], F32)
        bkk = consts.tile([128, 1], F32)
        bcv = consts.tile([64, 1], F32)
        bov2 = consts.tile([128, 1], F32)
        dvv2 = consts.tile([128, 1], F32)
        ones = consts.tile([128, 1], BF16)
        ident_sb = consts.tile([128, 64], BF16)

        nc.vector.memset(x2h[64:128, 4088:4096], 0.0)
        nc.vector.memset(x2w[64:128, 4024:4096], 0.0)
        nc.sync.dma_start(out=x2h[0:64, :], in_=xbf[:, :])
        nc.scalar.dma_start(out=x2h[64:128, 0:4095], in_=xbf[:, 1:4096])
        nc.sync.dma_start(out=wq_sb[:], in_=wqblk[:])
        nc.sync.dma_start(out=wk_sb[:], in_=wkblk[:])
        nc.scalar.dma_start(out=bqq[:], in_=bqq_d[:])
        nc.scalar.dma_start(out=bkk[:], in_=bkk_d[:])
        nc.sync.dma_start(out=wc_sb[:], in_=wcpt[:])
        nc.scalar.dma_start(out=x2w[0:64, :], in_=xbf[:, :])
        nc.sync.dma_start(out=x2w[64:128, 0:4032], in_=xbf[:, 64:4096])
        nc.sync.dma_start(out=woa_sb[:], in_=woa[:])
        nc.sync.dma_start(out=wob_sb[:], in_=wob[:])
        nc.sync.dma_start(out=bcv[:], in_=bc_d[:])
        nc.sync.dma_start(out=bov2[0:64], in_=bo_d[:])
        nc.sync.dma_start(out=bov2[64:128], in_=bo_d[:])
        nc.sync.dma_start(out=dvv2[0:64], in_=dv_d[:])
        nc.sync.dma_start(out=dvv2[64:128], in_=dv_d[:])
        nc.sync.dma_start(out=ident_sb[:], in_=ident_d[:])
        nc.vector.memset(ones[:], 1.0)

        # conv image buffers (zero borders)
        catA = consts.tile([128, PAD * PAD], BF16)  # rows 0-63 x, 64-127 h_att
        catB = consts.tile([128, PAD * PAD], BF16)  # rows 0-63 w_att, rest zero
        nc.gpsimd.memset(catA[:], 0.0)
        nc.gpsimd.memset(catB[:], 0.0)

        def pad_interior_ap(t, p0, p1, row0=0, nrows=L):
            base = t[p0:p1, :]
            return bass.AP(tensor=base.tensor,
                           offset=base.offset + (row0 + 1) * PAD + 1,
                           ap=[base.ap[0], [PAD, nrows], [1, L]])

        # x part of the conv image
        nc.sync.dma_start(out=pad_interior_ap(catA, 0, 64), in_=x2h[0:64, :])

        # ---- projections -------------------------------------------------
        # Per direction: Q [128, 32, 64] (block kb = spatial pair, partition
        # = (parity, channel)), Kdup [128, 32, 64, 2] (K duplicated pairs).
        q_t = {d: qk.tile([128, NKB, 64], BF16, tag=f"q{d}", name=f"q_{d}") for d in "hw"}
        kd_t = {d: qk.tile([128, NKB, 64, 2], BF16, tag=f"k{d}", name=f"kd_{d}") for d in "hw"}

        with tc.tile_pool(name="projps", bufs=6, space="PSUM") as pps:
            for d in "hw":
                x2 = x2h if d == "h" else x2w
                for t8 in range(4):  # 8 g-blocks per psum tile
                    for proj in "qk":
                        wsb = wq_sb if proj == "q" else wk_sb
                        bias = bqq if proj == "q" else bkk
                        ps = pps.tile([128, 8, 64], F32, tag="proj")
                        for half in range(2):  # 4 g per matmul
                            g0 = t8 * 8 + half * 4
                            if d == "h":
                                rhs = bass.AP(tensor=x2.tensor, offset=x2.offset + 2 * g0,
                                              ap=[x2.ap[0], [2, 4], [64, 64]])
                            else:
                                rhs = bass.AP(tensor=x2.tensor, offset=x2.offset + 128 * g0,
                                              ap=[x2.ap[0], [128, 4], [1, 64]])
                            nc.tensor.matmul(out=ps[:, half * 4:(half + 1) * 4, :],
                                             lhsT=wsb[:], rhs=rhs,
                                             start=True, stop=True)
                        if proj == "q":
                            nc.scalar.activation(
                                out=q_t[d][:, t8 * 8:(t8 + 1) * 8, :], in_=ps[:],
                                func=mybir.ActivationFunctionType.Identity,
                                bias=bias[:], scale=1.0)
                        else:
                            for dup in range(2):
                                dst = bass.AP(
                                    tensor=kd_t[d].tensor,
                                    offset=kd_t[d].offset + t8 * 8 * 128 + dup,
                                    ap=[kd_t[d].ap[0], [128, 8], [2, 64]])
                                nc.scalar.activation(
                                    out=dst, in_=ps[:],
                                    func=mybir.ActivationFunctionType.Identity,
                                    bias=bias[:], scale=1.0)

        # ---- attention + softmax + apply + conv --------------------------
        # att padded to 128 partitions (rows 64-127 zero) so the Z column
        # sums run as full-width K=128 matmuls.
        att_t = {}
        for d in "hw":
            att_t[d] = work.tile([128, L, L], BF16, tag=f"att{d}", bufs=1, name=f"att_{d}")
            nc.vector.memset(att_t[d][64:128, :, :], 0.0)
        hat_t = {d: work.tile([64, N], BF16, tag=f"hat{d}", bufs=1, name=f"hat_{d}") for d in "hw"}

        def conv_tail(cps, rps):
            # 3x3 conv: both images accumulate into one psum pass per chunk
            for rp in rps:
                cv = cps.tile([128, 512], F32, tag="cv", name="cv")
                for tap in range(9):
                    dy, dx = tap // 3, tap % 3
                    for half in range(2):
                        r = rp * 2 + half
                        off = (r * 8 + dy) * PAD + dx
                        for src_sb, w_sb, part in ((catA, woa_sb, 0), (catB, wob_sb, 1)):
                            rhs = bass.AP(tensor=src_sb.tensor, offset=src_sb.offset + off,
                                          ap=[src_sb.ap[0], [PAD, 8], [1, 64]])
                            nc.tensor.matmul(out=cv[half * 64:(half + 1) * 64, :],
                                             lhsT=w_sb[:, tap, :], rhs=rhs,
                                             start=(tap == 0 and part == 0),
                                             stop=(tap == 8 and part == 1),
                                             skip_group_check=True,
                                             tile_position=(0, half * 64))
                ysb = work.tile([128, 512], F32, tag="ysb", name="ysb")
                nc.scalar.activation(out=ysb[0:64, :], in_=cv[0:64, :],
                                     func=mybir.ActivationFunctionType.Relu,
                                     bias=bov2[0:64], scale=1.0)
                nc.scalar.activation(out=ysb[64:128, :], in_=cv[64:128, :],
                                     func=mybir.ActivationFunctionType.Relu,
                                     bias=bov2[64:128], scale=1.0)
                nc.vector.tensor_scalar_add(out=ysb[:], in0=ysb[:], scalar1=dvv2[:])
                nc.sync.dma_start(out=y[:, (2 * rp) * 512:(2 * rp + 1) * 512],
                                  in_=ysb[0:64, :])
                nc.sync.dma_start(out=y[:, (2 * rp + 1) * 512:(2 * rp + 2) * 512],
                                  in_=ysb[64:128, :])

        with tc.tile_pool(name="ahps", bufs=1, space="PSUM") as aps, \
             tc.tile_pool(name="zps", bufs=2, space="PSUM") as zps, \
             tc.tile_pool(name="cvps", bufs=2, space="PSUM") as cps:
            for d in "hw":
                q, kd = q_t[d], kd_t[d]
                for jh in range(2):  # column halves (j in [jh*32, jh*32+32))
                    ah = aps.tile([128, 2048], F32, tag="ah", name="ah")
                    for kbp in range(NKB // 2):
                        grhs = {}
                        for half in range(2):
                            kb = kbp * 2 + half
                            g = gpool.tile([128, 32, 64], BF16, tag="g", name=f"g{half}")
                            # G[k, j, i] = K[k,j] * Q[k,i] (2x-mode paired APs)
                            in0 = bass.AP(
                                tensor=kd.tensor,
                                offset=kd.offset + kb * 128 + jh * 64,
                                ap=[kd.ap[0], [2, 32], [0, 32], [1, 2]])
                            in1 = bass.AP(
                                tensor=q.tensor, offset=q.offset + kb * 64,
                                ap=[q.ap[0], [0, 32], [2, 32], [1, 2]])
                            gout = bass.AP(
                                tensor=g.tensor, offset=g.offset,
                                ap=[g.ap[0], [64, 32], [2, 32], [1, 2]])
                            nc.vector.tensor_mul(out=gout, in0=in0, in1=in1)
                            grhs[half] = g[:].rearrange("p a b -> p (a b)")
                        for ns in range(4):
                            for half in range(2):
                                kb = kbp * 2 + half
                                nc.tensor.matmul(
                                    out=ah[half * 64:(half + 1) * 64, ns * 512:(ns + 1) * 512],
                                    lhsT=wc_sb[:, kb, :],
                                    rhs=grhs[half][:, ns * 512:(ns + 1) * 512],
                                    start=(kbp == 0),
                                    stop=(kbp == NKB // 2 - 1 and ns == 3),
                                    skip_group_check=True,
                                    tile_position=(0, half * 64))
                    # fold the odd-half partial into the even-half region via
                    # an identity matmul (ACT copy to SBUF, same partitions).
                    fold = work.tile([128, 2048], BF16, tag="fold", name="fold", bufs=2)
                    nc.scalar.copy(out=fold[64:128, :], in_=ah[64:128, :])
                    for ns in range(4):
                        nc.tensor.matmul(
                            out=ah[0:64, ns * 512:(ns + 1) * 512],
                            lhsT=ident_sb[64:128, :],
                            rhs=fold[64:128, ns * 512:(ns + 1) * 512],
                            start=False, stop=True,
                            skip_group_check=True,
                            tile_position=(64, 0))
                    # exp with transposed read: ah[(j,i)] -> att[(i, j)]
                    src = bass.AP(tensor=ah.tensor, offset=ah.offset,
                                  ap=[[ah.ap[0][0], 64], [1, 64], [64, 32]])
                    nc.scalar.activation(
                        out=att_t[d][0:64, :, jh * 32:(jh + 1) * 32], in_=src,
                        func=mybir.ActivationFunctionType.Exp,
                        bias=bcv[:], scale=1.0)

                att = att_t[d][:].rearrange("p a b -> p (a b)")
                if debug:
                    tp = taps["t_att"] if d == "h" else taps["t_z"]
                    nc.sync.dma_start(out=tp[:], in_=att[0:64, :])
                # column sums Z via ones-matmul (K=128; att rows 64-127 zero),
                # then spread Z across 64 lanes via DRAM for the reciprocal
                zrow = work.tile([1, N], F32, tag="zrow", bufs=1)
                for col in range(8):
                    zt = zps.tile([1, 512], F32, tag="z", name="zt")
                    nc.tensor.matmul(out=zt[:], lhsT=ones[:],
                                     rhs=att[:, col * 512:(col + 1) * 512],
                                     start=True, stop=True)
                    nc.scalar.copy(out=zrow[:, col * 512:(col + 1) * 512], in_=zt[:])
                zs = work.tile([64, 64], F32, tag="zs", bufs=1)
                rzs = work.tile([64, 64], BF16, tag="rzs", bufs=1)
                nc.sync.dma_start(out=zs[:], in_=zrow[:])
                with nc.allow_low_precision(reason="1/Z multiplier in bf16"):
                    nc.vector.reciprocal(out=rzs[:], in_=zs[:])
                rz = dpool.tile([64, 64], BF16, tag="rzd2")
                nc.sync.dma_start(out=rz[:], in_=rzs[:])
                rzb = work.tile([64, N], BF16, tag="rzb", bufs=1)
                for ch in range(2):
                    sl = slice(ch * 2048, (ch + 1) * 2048)
                    nc.sync.dma_start(
                        out=rzb[:, sl],
                        in_=bass.AP(tensor=rz.tensor, offset=rz.offset + ch * 2048,
                                    ap=[[0, 64], [64, 32], [1, 64]]))
                # h_att = x * att * (1/Z); write into the padded conv image
                tmp = work.tile([64, N], BF16, tag="tmp", bufs=1)
                nc.vector.tensor_mul(out=tmp[:], in0=att[0:64, :], in1=x2h[0:64, :])
                cat_dst, cat_p0 = (catA, 64) if d == "h" else (catB, 0)
                if d == "h":
                    for ch in range(2):
                        sl = slice(ch * 2048, (ch + 1) * 2048)
                        nc.vector.tensor_mul(out=hat_t[d][:, sl], in0=tmp[:, sl],
                                             in1=rzb[:, sl])
                        nc.sync.dma_start(
                            out=pad_interior_ap(cat_dst, cat_p0, cat_p0 + 64,
                                                row0=ch * 32, nrows=32),
                            in_=hat_t[d][:, sl])
                else:
                    # 4 row-band chunks, written straight into the padded
                    # image (strided dst); conv row-pair rp needs image rows
                    # up to 16(rp+1), so emit conv rp-1 after each chunk to
                    # pipeline the conv with the apply.
                    for ch in range(4):
                        sl = slice(ch * 1024, (ch + 1) * 1024)
                        tv = tmp[:, sl].rearrange("p (a b) -> p a b", b=64)
                        rv = rzb[:, sl].rearrange("p (a b) -> p a b", b=64)
                        nc.vector.tensor_mul(
                            out=pad_interior_ap(cat_dst, cat_p0, cat_p0 + 64,
                                                row0=ch * 16, nrows=16),
                            in0=tv, in1=rv)
                        if ch >= 1:
                            conv_tail(cps, [ch - 1])
                    conv_tail(cps, [3])

        if debug:
            nc.sync.dma_start(out=taps["t_hatt"][:], in_=hat_t["h"][:])
            nc.sync.dma_start(out=taps["t_watt"][:], in_=hat_t["w"][:])

    nc.finalize()
    return nc


def _host_prep(Wq, bq, Wk, bk, Wc, bc, Wo, bo, gamma, beta, run_mean, run_var):
    bf = ml_dtypes.bfloat16
    wqblk = np.zeros((128, 128), np.float32)
    wqblk[0:64, 0:64] = Wq.T
    wqblk[64:128, 64:128] = Wq.T
    wkblk = np.zeros((128, 128), np.float32)
    wkblk[0:64, 0:64] = Wk.T
    wkblk[64:128, 64:128] = Wk.T
    # Wc permuted so the contraction index is (spatial, channel)
    wcp = Wc.reshape(C, C, L).transpose(0, 2, 1).reshape(C, C * L)
    wcpt = np.ascontiguousarray(
        wcp.T.reshape(NKB, 128, 64).transpose(1, 0, 2))  # [128, 32, 64]
    inv = gamma / np.sqrt(run_var + BN_EPS)
    wo_eff = Wo * inv[:, None, None, None]
    wot = wo_eff.transpose(1, 2, 3, 0).reshape(3 * C, 9, C)  # [192, 9, 64]
    zpad = np.zeros((64, 9, C), np.float32)
    return {
        "wqblk": wqblk.astype(bf), "wkblk": wkblk.astype(bf),
        "wcpt": wcpt.astype(bf),
        "woa": np.ascontiguousarray(wot[0:128]).astype(bf),
        "wob": np.concatenate([wot[128:192], zpad]).astype(bf),
        "bqq": np.concatenate([bq, bq]).reshape(128, 1).astype(np.float32),
        "bkk": np.concatenate([bk, bk]).reshape(128, 1).astype(np.float32),
        "bc": bc.reshape(64, 1).astype(np.float32),
        "bo_eff": (bo * inv).reshape(64, 1).astype(np.float32),
        "d_vec": (beta - run_mean * inv).reshape(64, 1).astype(np.float32),
        "ident": np.concatenate([np.zeros((64, 64), np.float32),
                                 np.eye(64, dtype=np.float32)]).astype(bf),
    }


def kernel(x, Wq, bq, Wk, bk, Wc, bc, Wo, bo, gamma, beta, run_mean, run_var,
           debug=False, trace=False, trace_kwargs=None):
    x = np.asarray(x, np.float32)
    weights = _host_prep(
        np.asarray(Wq, np.float32), np.asarray(bq, np.float32),
        np.asarray(Wk, np.float32), np.asarray(bk, np.float32),
        np.asarray(Wc, np.float32), np.asarray(bc, np.float32),
        np.asarray(Wo, np.float32), np.asarray(bo, np.float32),
        np.asarray(gamma, np.float32), np.asarray(beta, np.float32),
        np.asarray(run_mean, np.float32), np.asarray(run_var, np.float32))
    key = bool(debug)
    if key not in _CACHE:
        _CACHE[key] = _build_nc(debug=debug)
    nc = _CACHE[key]
    bf = ml_dtypes.bfloat16
    in_maps = []
    for b in range(B):
        m = dict(weights)
        m["xbf"] = np.ascontiguousarray(x[b].reshape(C, N)).astype(bf)
        in_maps.append(m)
    kwargs = {}
    if trace:
        kwargs = dict(trace=True, trace_cores=[0], **(trace_kwargs or {}))
    res = run_bass_kernel_spmd(nc, in_maps, core_ids=list(range(B)), **kwargs)
    out = np.stack([res.results[b]["y"].reshape(C, L, L) for b in range(B)])
    if debug or trace:
        return out, res
    return out
